# revision 36
# baseline (speedup 1.0000x reference)
"""Trainium2 Bass kernel: Sudoku information gain H(before) - H(after).

Self-contained: builds one SPMD Bass/Tile program, shards the batch
across 8 NeuronCores (pure data parallel), runs via
run_bass_kernel_spmd, and reassembles the full [B] output.

Algorithm per 9x9 grid (values 0..9, 0 = empty):
  encode each cell x as e = 1024 >> x on ScalarE (Exp activation):
    bit 10 <=> empty, bit (10-v) <=> value v.
  Bitwise-OR reductions of e give row/col/box presence masks (VectorE,
  uint16, 2x/4x-mode ops).  Per cell m = row|col|box, with the q->qx
  broadcast materialized on the Pool engine so the final OR is a packed
  2x op.  SWAR popcount: pair counts (h,g,s), base-16 fold (a2,c0,c),
  digit sum via mod-15 (q15,v15,t1 — the u16 multiply saturates on
  TRN2, so the wrap-multiply shortcut is unusable).  fw = 8*[empty]
  comes from a ScalarE Relu, u = relu(fw - popcount) too, and
  Ln(u+1) = ln(max(9-forbidden,1)) for empty cells / 0 for filled ones.
  The per-grid 81-way sum runs as an in-place fold tree on the Pool
  engine (f32); before/after are subtracted and scaled there as well.

Engine balance per tile: DVE does the OR tree + SWAR chain, ScalarE
does Exp/fw/rowx/relu/Ln, Pool does the qx replication + fold tree,
all software-pipelined with a one-tile skew (pre stage: DMA/encode/
masks; main stage: combine/chain/Ln/fold) so the in-order DVE queue
never waits on another engine.  The first tile's pre and the last
tile's main run in half-F parts to shorten the pipeline head/tail.

One explicit InstLoadActFuncSet pins the shared exp+ln activation
table (natural_log_exp_and_others, which also holds relu/copy), so no
per-activation table reloads occur.
"""

import math
from contextlib import ExitStack

import numpy as np

import concourse.bass as bass
import concourse.bacc as bacc
import concourse.tile as tile
from concourse import mybir
from concourse.alu_op_type import AluOpType
from concourse.bass_utils import run_bass_kernel_spmd

F32 = mybir.dt.float32
U16 = mybir.dt.uint16
I16 = mybir.dt.int16

LN2 = math.log(2.0)
LOG1024 = math.log(1024.0)
EPS = 1e-5

OR = AluOpType.bitwise_or
AND = AluOpType.bitwise_and
ADD = AluOpType.add
SUB = AluOpType.subtract
MULT = AluOpType.mult
MAX = AluOpType.max
SHR = AluOpType.logical_shift_right

N_CORES = 8
BATCH = 262144
PER_CORE = BATCH // N_CORES  # 32768
F = 32  # grids per partition per tile
MOD_BIAS = -0.4999  # subtracted before the int16 convert in floor(c/15)
ACT_TABLE_BOTH = 6  # act_func_set_id of natural_log_exp_and_others


def _masks(nc, wp, es, nb8, F, Fa=None):
    """Unit masks through qx for BOTH tensors, ops interleaved pairwise.

    Each cross-engine hop (DVE -> Pool transpose -> DVE, DVE -> Act
    replicate -> DVE) is covered by the sibling tensor's independent
    work so the in-order DVE queue never stalls.  Also emits
    fw = 8*[cell empty] on the Act engine (consumed late, in the main
    stage).  Returns {key: (qx, col, fw)}.
    """
    if Fa is None:
        Fa = F
    t3s, rows, bcols, cols, boxes, rowxs, qs, qxs, fws = ({} for _ in range(9))

    for k, e in es.items():
        e4 = e.rearrange("p (f r c) -> p f r c", f=F, r=9, c=9)
        t3 = wp.tile([128, Fa * 27], U16, tag="t3", bufs=3, name=f"t3{k}")[:][
            :, : F * 27
        ]
        t3v = t3.rearrange("p (f r b) -> p f r b", f=F, r=9, b=3)
        nc.vector.tensor_tensor(t3v, e4[:, :, :, 0:3], e4[:, :, :, 3:6], op=OR)
        nc.vector.tensor_tensor(t3v, t3v, e4[:, :, :, 6:9], op=OR)
        t3s[k] = t3

    for k, e in es.items():
        e5 = e.rearrange("p (f b i c) -> p f b i c", f=F, b=3, i=3, c=9)
        bcol = wp.tile([128, Fa * 27], U16, tag="bcol", bufs=3, name=f"bcol{k}")[:][
            :, : F * 27
        ]
        bv = bcol.rearrange("p (f b c) -> p f b c", f=F, b=3, c=9)
        nc.vector.tensor_tensor(bv, e5[:, :, :, 0, :], e5[:, :, :, 1, :], op=OR)
        nc.vector.tensor_tensor(bv, bv, e5[:, :, :, 2, :], op=OR)
        bcols[k] = bcol

    for k in es:
        col = wp.tile([128, Fa * 9], U16, tag="col", bufs=3, name=f"col{k}")[:][
            :, : F * 9
        ]
        cv = col.rearrange("p (f c) -> p f c", f=F, c=9)
        bc3 = bcols[k].rearrange("p (f b c) -> p f b c", f=F, b=3, c=9)
        nc.vector.tensor_tensor(cv, bc3[:, :, 0, :], bc3[:, :, 1, :], op=OR)
        nc.vector.tensor_tensor(cv, cv, bc3[:, :, 2, :], op=OR)
        cols[k] = col

    for k in es:
        row = wp.tile([128, Fa * 9], U16, tag="row", bufs=3, name=f"row{k}")[:][
            :, : F * 9
        ]
        rv = row.rearrange("p (f r) -> p f r", f=F, r=9)
        t3b = t3s[k].rearrange("p (f r b) -> p f r b", f=F, r=9, b=3)
        nc.vector.tensor_tensor(rv, t3b[:, :, :, 0], t3b[:, :, :, 1], op=OR)
        nc.vector.tensor_tensor(rv, rv, t3b[:, :, :, 2], op=OR)
        rows[k] = row

    for k in es:
        # rowx[f, (b,i), k] = row[f, (b,i)]: replicate on Act (stride-0)
        rowx = wp.tile([128, Fa * 27], U16, tag="rowx", bufs=3, name=f"rowx{k}")[:][
            :, : F * 27
        ]
        rowxv = rowx.rearrange("p (u k) -> p u k", u=F * 9, k=3)
        nc.scalar.copy(rowxv, rows[k].unsqueeze(2).broadcast_to((128, F * 9, 3)))
        rowxs[k] = rowx

    for k in es:
        box = wp.tile([128, Fa * 9], U16, tag="box", bufs=3, name=f"box{k}")[:][
            :, : F * 9
        ]
        xv3 = box.rearrange("p (f b k) -> p f b k", f=F, b=3, k=3)
        bc4 = bcols[k].rearrange("p (f b k i) -> p f b k i", f=F, b=3, k=3, i=3)
        nc.vector.tensor_tensor(xv3, bc4[:, :, :, :, 0], bc4[:, :, :, :, 1], op=OR)
        nc.vector.tensor_tensor(xv3, xv3, bc4[:, :, :, :, 2], op=OR)
        boxes[k] = box.rearrange("p (g k) -> p g k", g=F * 3, k=3)

    for k in es:
        # q[f, (b,i), bc] = rowx | box (box bcast over i sits on a
        # middle dim, so every AP keeps a packed innermost -> 2x mode)
        q = wp.tile([128, Fa * 27], U16, tag="q", bufs=3, name=f"q{k}")[:][
            :, : F * 27
        ]
        qv = q.rearrange("p (g i k) -> p g i k", g=F * 3, i=3, k=3)
        nc.vector.tensor_tensor(
            qv,
            rowxs[k].rearrange("p (g i k) -> p g i k", g=F * 3, i=3, k=3),
            boxes[k].unsqueeze(2).broadcast_to((128, F * 3, 3, 3)),
            op=OR,
        )
        qs[k] = q

    for k in es:
        # qx[f, r, c] = q[f, r, bc(c)]: replicate 3x innermost on Pool
        # (stride-0 reads are legal there).  m = qx | col then runs as
        # one packed 2x DVE op instead of three 1x broadcast ops.
        qx = wp.tile([128, Fa * 81], U16, tag="qx", name=f"qx{k}")[:][
            :, : F * 81
        ]
        qxv = qx.rearrange("p (u i) -> p u i", u=F * 27, i=3)
        qu = qs[k].rearrange("p (u) -> p u", u=F * 27)
        nc.gpsimd.tensor_copy(qxv, qu.unsqueeze(2).broadcast_to((128, F * 27, 3)))
        qxs[k] = qx

    for k, e in es.items():
        # fw = relu(e/64 - 8): 1024 (empty) -> 8, every value bit -> 0.
        fw = wp.tile([128, Fa * 81], U16, tag="fw", name=f"fw{k}")[:][
            :, : F * 81
        ]
        nc.scalar.activation(
            fw, e[:], mybir.ActivationFunctionType.Relu, bias=nb8, scale=1.0 / 64.0
        )
        fws[k] = fw

    return {k: (qxs[k], cols[k], fws[k]) for k in es}


def _combine(nc, wp, qx_s, col_s, Fp, Fa):
    """m = qx | col on a sub-range of Fp grids (tile allocated at Fa)."""
    m_t = wp.tile([128, Fa * 81], U16, tag="m", bufs=2)
    m = m_t[:][:, : Fp * 81]
    mv3 = m.rearrange("p (f r c) -> p f r c", f=Fp, r=9, c=9)
    colb = col_s.rearrange("p (f c) -> p f c", f=Fp, c=9)
    nc.vector.tensor_tensor(
        mv3,
        qx_s.rearrange("p (f r c) -> p f r c", f=Fp, r=9, c=9),
        colb.unsqueeze(2).broadcast_to((128, Fp, 9, 9)),
        op=OR,
    )
    return m


def _entropy_u(nc, wp, m, fw_s, zb, Fp, Fa):
    """u(i16) with Ln(u+1) = per-cell entropy contribution.

    SWAR popcount of m's bits 1..9 (bit 10 never enters: the 0x155/0x55
    masks skip it), mod-15 digit sum (u16 multiply saturates on TRN2 so
    the wrap-multiply shortcut is unusable), fused with the own-cell
    empty gate fw.  Buffers A/B/C and m are reused in place across
    chain stages.
    """
    n = Fp * 81
    A = wp.tile([128, Fa * 81], U16, tag="A", bufs=2, name="A")[:][:, :n]
    B = wp.tile([128, Fa * 81], U16, tag="B", bufs=2, name="B")[:][:, :n]
    C = wp.tile([128, Fa * 81], U16, tag="C", bufs=2, name="C")[:][:, :n]

    h, g = A, B
    nc.vector.tensor_scalar(h, m, 1, 0x155, op0=SHR, op1=AND)
    nc.vector.tensor_scalar(g, m, 2, 0x55, op0=SHR, op1=AND)
    s = m  # m dead after h,g
    nc.vector.tensor_tensor(s, h, g, op=ADD)
    a2, c0 = A, B  # h,g consumed
    nc.vector.tensor_scalar(a2, s, 2, 0x33, op0=SHR, op1=AND)
    nc.vector.tensor_scalar(c0, s, 0x333, None, op0=AND)
    c = C
    nc.vector.tensor_tensor(c, c0, a2, op=ADD)
    # c = f0 + 16*f1 + 256*f2 with f0,f1<=4, f2<=1; popcount = c mod 15
    q15 = A.bitcast(I16)  # a2 consumed
    nc.vector.tensor_scalar(q15, c, 1.0 / 15.0, MOD_BIAS, op0=MULT, op1=ADD)
    v15 = B.bitcast(I16)  # c0 consumed
    nc.vector.tensor_scalar(v15, q15, 15, None, op0=MULT)
    t1 = m.bitcast(I16)  # s dead after a2,c0
    nc.vector.tensor_tensor(t1, c, v15, op=SUB)
    t = A.bitcast(I16)  # q15 dead
    nc.vector.tensor_tensor(t, t1, fw_s.bitcast(I16), op=SUB)
    # u = relu(-t) runs on the Activation engine (chain sink: it feeds
    # only the Ln on the same engine, so the DVE queue never waits)
    u = B.bitcast(I16)  # v15 dead
    nc.scalar.activation(u, t, mybir.ActivationFunctionType.Relu, bias=zb, scale=-1.0)
    return u


def _emit(tc, out_ap, gb_ap, ga_ap, n_grids, F):
    nc = tc.nc
    per_tile = 128 * F
    n_tiles = n_grids // per_tile

    # Pin the activation table that contains BOTH exp and ln: without
    # this the table-load pass alternates exp/ln tables (1.3us each).
    ld = mybir.InstLoadActFuncSet(
        name=nc.get_next_instruction_name(),
        act_func_set_id=ACT_TABLE_BOTH,
        ins=[],
        outs=[],
    )
    nc.scalar.add_instruction(ld)

    with ExitStack() as ctx:
        cp = ctx.enter_context(tc.tile_pool(name="const", bufs=1))
        iop = ctx.enter_context(tc.tile_pool(name="io", bufs=3))
        wp = ctx.enter_context(tc.tile_pool(name="work", bufs=4))
        accp = ctx.enter_context(tc.tile_pool(name="acc", bufs=3))

        enc_bias = cp.tile([128, 1], F32, tag="enc_bias")
        nc.vector.memset(enc_bias[:], LOG1024 + EPS)
        nb8 = cp.tile([128, 1], F32, tag="nb8")
        nc.vector.memset(nb8[:], -8.0)
        zb = cp.tile([128, 1], F32, tag="zb")
        nc.vector.memset(zb[:], 0.0)

        state = {}

        def pre(i):
            """DMA + encode + mask build through qx for tile i.

            Tile 0 runs in two half-F parts so the DVE starts working
            after half a DMA+Exp instead of a full one (pipeline head).
            """
            parts = [(0, F)] if i > 0 else [(0, F // 2), (F // 2, F)]
            out = []
            for f0, f1 in parts:
                Fp = f1 - f0
                es = {}
                for key, src in (("b", gb_ap), ("a", ga_ap)):
                    x = iop.tile([128, F * 81], F32, tag="x", name=f"x{key}")
                    xs = x[:][:, : Fp * 81]
                    view = src[i * per_tile : (i + 1) * per_tile, :].rearrange(
                        "(p f) c -> p (f c)", p=128
                    )
                    nc.sync.dma_start(xs, view[:, f0 * 81 : f1 * 81])
                    e = wp.tile([128, F * 81], U16, tag="e", name=f"e{key}")
                    es[key] = e[:][:, : Fp * 81]
                    nc.scalar.activation(
                        es[key],
                        xs,
                        mybir.ActivationFunctionType.Exp,
                        bias=enc_bias[:],
                        scale=-LN2,
                    )
                out.append(((f0, f1), _masks(nc, wp, es, nb8, Fp, F)))
            state[i] = out

        def main(i):
            """Per-cell mask | col, SWAR chain, Ln, Pool fold for tile i.

            The last tile runs in two half-F parts so the pipeline drain
            tail (Ln + fold tree + out-DMA after the final DVE op) is
            half as long.
            """
            pre_parts = state.pop(i)
            work = []
            for (pf0, pf1), st in pre_parts:
                if i == n_tiles - 1 and pf1 - pf0 > F // 2:
                    mid = (pf0 + pf1) // 2
                    work.append(((pf0, mid), st, pf0))
                    work.append(((mid, pf1), st, pf0))
                else:
                    work.append(((pf0, pf1), st, pf0))
            for (f0, f1), st, base in work:
                Fp = f1 - f0
                tots = {}
                for key in ("b", "a"):
                    qx, col, fw = st[key]
                    qx_s = qx[:, (f0 - base) * 81 : (f1 - base) * 81]
                    col_s = col[:, (f0 - base) * 9 : (f1 - base) * 9]
                    fw_s = fw[:, (f0 - base) * 81 : (f1 - base) * 81]
                    m = _combine(nc, wp, qx_s, col_s, Fp, F)
                    u = _entropy_u(nc, wp, m, fw_s, zb, Fp, F)
                    lnv = wp.tile([128, F * 81], F32, tag="lnv", bufs=2)
                    lnv_s = lnv[:][:, : Fp * 81]
                    nc.scalar.activation(
                        lnv_s, u, mybir.ActivationFunctionType.Ln, bias=1.0
                    )
                    # Per-grid sum of the 81 ln values entirely on the
                    # Pool engine: in-place 81->27->9->3->1 fold tree
                    # (Pool only supports f32 arithmetic).
                    lv = lnv_s.rearrange("p (f c) -> p f c", f=Fp, c=81)
                    for width in (27, 9, 3, 1):
                        nc.gpsimd.tensor_tensor(
                            lv[:, :, 0:width],
                            lv[:, :, 0:width],
                            lv[:, :, width : 2 * width],
                            op=ADD,
                        )
                        nc.gpsimd.tensor_tensor(
                            lv[:, :, 0:width],
                            lv[:, :, 0:width],
                            lv[:, :, 2 * width : 3 * width],
                            op=ADD,
                        )
                    tots[key] = lv[:, :, 0]

                diff = accp.tile([128, F], F32, tag="diff")
                diff_s = diff[:][:, :Fp]
                nc.gpsimd.tensor_tensor(diff_s, tots["b"], tots["a"], op=SUB)
                nc.gpsimd.tensor_scalar(diff_s, diff_s, 1.0 / LN2, None, op0=MULT)
                out_view = out_ap[i * per_tile : (i + 1) * per_tile].rearrange(
                    "(p f) -> p f", p=128
                )
                nc.sync.dma_start(out_view[:, f0:f1], diff_s)

        # one-tile software pipeline skew: tile i's cross-engine mask
        # staging (Act transposes/replications) completes while the DVE
        # drains tile i-1's long chain, so the in-order DVE queue never
        # stalls on the Activation engine.
        for i in range(n_tiles + 1):
            if i < n_tiles:
                pre(i)
            if i >= 1:
                main(i - 1)


_PROGRAM_CACHE = {}


def _build_program():
    key = (PER_CORE, F)
    if key in _PROGRAM_CACHE:
        return _PROGRAM_CACHE[key]
    nc = bacc.Bacc("TRN2", target_bir_lowering=False, debug=False)
    gb = nc.dram_tensor("grid_before", [PER_CORE, 81], F32, kind="ExternalInput")
    ga = nc.dram_tensor("grid_after", [PER_CORE, 81], F32, kind="ExternalInput")
    out = nc.dram_tensor("out", [PER_CORE], F32, kind="ExternalOutput")
    with tile.TileContext(nc) as tc:
        _emit(tc, out.ap(), gb.ap(), ga.ap(), PER_CORE, F)
    nc.finalize()
    _PROGRAM_CACHE[key] = nc
    return nc


def run(grid_before, grid_after, trace=False, **trace_kwargs):
    gb = np.ascontiguousarray(
        np.asarray(grid_before, dtype=np.float32).reshape(BATCH, 81)
    )
    ga = np.ascontiguousarray(
        np.asarray(grid_after, dtype=np.float32).reshape(BATCH, 81)
    )
    nc = _build_program()
    in_maps = [
        {
            "grid_before": gb[k * PER_CORE : (k + 1) * PER_CORE],
            "grid_after": ga[k * PER_CORE : (k + 1) * PER_CORE],
        }
        for k in range(N_CORES)
    ]
    res = run_bass_kernel_spmd(
        nc, in_maps, list(range(N_CORES)), trace=trace, **trace_kwargs
    )
    out = np.concatenate([res.results[k]["out"] for k in range(N_CORES)])
    return out, res


def kernel(grid_before, grid_after):
    out, _ = run(grid_before, grid_after)
    return out


def bench(grid_before, grid_after, iters=12, warmup=3):
    """Time repeated executions with device-resident inputs.

    Mirrors bass2jax.run_bass_via_pjrt's shard_map structure but keeps the
    170MB of inputs on the devices between iterations, so the measured
    per-iteration wall time approximates kernel execution + dispatch.
    """
    import time

    import jax
    import concourse.mybir as mybir_
    from jax.sharding import Mesh, NamedSharding, PartitionSpec
    from jax.experimental.shard_map import shard_map
    from concourse.bass2jax import (
        _bass_exec_p,
        install_neuronx_cc_hook,
        partition_id_tensor,
    )

    install_neuronx_cc_hook()
    gb = np.ascontiguousarray(
        np.asarray(grid_before, dtype=np.float32).reshape(BATCH, 81)
    )
    ga = np.ascontiguousarray(
        np.asarray(grid_after, dtype=np.float32).reshape(BATCH, 81)
    )
    nc = _build_program()

    part_name = nc.partition_id_tensor.name if nc.partition_id_tensor else None
    in_names, out_names, out_avals, zero_outs = [], [], [], []
    for alloc in nc.m.functions[0].allocations:
        if not isinstance(alloc, mybir.MemoryLocationSet):
            continue
        name = alloc.memorylocations[0].name
        if alloc.kind == "ExternalInput":
            if name != part_name:
                in_names.append(name)
        elif alloc.kind == "ExternalOutput":
            out_names.append(name)
            shape = tuple(alloc.tensor_shape)
            dtype = mybir_.dt.np(alloc.dtype)
            out_avals.append(jax.core.ShapedArray(shape, dtype))
            zero_outs.append(np.zeros((N_CORES * shape[0], *shape[1:]), dtype))
    n_params = len(in_names)
    all_names = in_names + out_names
    if part_name is not None:
        all_names = all_names + [part_name]

    def _body(*args):
        operands = list(args)
        if part_name is not None:
            operands.append(partition_id_tensor())
        outs = _bass_exec_p.bind(
            *operands,
            out_avals=tuple(out_avals),
            in_names=tuple(all_names),
            out_names=tuple(out_names),
            lowering_input_output_aliases=(),
            sim_require_finite=True,
            sim_require_nnan=True,
            nc=nc,
        )
        return tuple(outs)

    devices = jax.devices()[:N_CORES]
    mesh = Mesh(np.asarray(devices), ("core",))
    spec = NamedSharding(mesh, PartitionSpec("core"))
    sharded = jax.jit(
        shard_map(
            _body,
            mesh=mesh,
            in_specs=(PartitionSpec("core"),) * (n_params + len(out_names)),
            out_specs=(PartitionSpec("core"),) * len(out_names),
            check_rep=False,
        ),
        keep_unused=True,
    )
    host_in = {"grid_before": gb, "grid_after": ga}
    dev_in = [jax.device_put(host_in[nm], spec) for nm in in_names]
    dev_zero = [jax.device_put(z, spec) for z in zero_outs]

    for _ in range(warmup):
        outs = sharded(*dev_in, *dev_zero)
    jax.block_until_ready(outs)
    t0 = time.perf_counter()
    for _ in range(iters):
        outs = sharded(*dev_in, *dev_zero)
    jax.block_until_ready(outs)
    t1 = time.perf_counter()
    per_iter_ns = (t1 - t0) / iters * 1e9
    out = np.asarray(outs[0])
    return per_iter_ns, out


# revision 41
# speedup vs baseline: 1.0362x; 1.0362x over previous
"""Trainium2 Bass kernel: Sudoku information gain H(before) - H(after).

Self-contained: builds one SPMD Bass/Tile program, shards the batch
across 8 NeuronCores (pure data parallel), runs via
run_bass_kernel_spmd, and reassembles the full [B] output.

Algorithm per 9x9 grid (values 0..9, 0 = empty):
  encode each cell x as e = 1024 >> x on ScalarE (Exp activation):
    bit 10 <=> empty, bit (10-v) <=> value v.
  Bitwise-OR reductions of e give row/col/box presence masks (VectorE,
  uint16, 2x/4x-mode ops).  Per cell m = row|col|box, with the q->qx
  broadcast materialized on the Pool engine so the final OR is a packed
  2x op.  SWAR popcount: pair counts (h,g,s), base-16 fold (a2,c0,c),
  digit sum via mod-15 (q15,v15,t1 — the u16 multiply saturates on
  TRN2, so the wrap-multiply shortcut is unusable).  fw = 8*[empty]
  comes from a ScalarE Relu, u = relu(fw - popcount) too, and
  Ln(u+1) = ln(max(9-forbidden,1)) for empty cells / 0 for filled ones.
  The per-grid 81-way sum runs as an in-place fold tree on the Pool
  engine (f32); before/after are subtracted and scaled there as well.

Engine balance per tile: DVE does the OR tree + SWAR chain, ScalarE
does Exp/fw/rowx/relu/Ln, Pool does the qx replication + fold tree,
all software-pipelined with a one-tile skew (pre stage: DMA/encode/
masks; main stage: combine/chain/Ln/fold) so the in-order DVE queue
never waits on another engine.  The first tile's pre and the last
tile's main run in half-F parts to shorten the pipeline head/tail.

One explicit InstLoadActFuncSet pins the shared exp+ln activation
table (natural_log_exp_and_others, which also holds relu/copy), so no
per-activation table reloads occur.
"""

import math
from contextlib import ExitStack

import numpy as np

import concourse.bass as bass
import concourse.bacc as bacc
import concourse.tile as tile
from concourse import mybir
from concourse.alu_op_type import AluOpType
from concourse.bass_utils import run_bass_kernel_spmd

F32 = mybir.dt.float32
U16 = mybir.dt.uint16
I16 = mybir.dt.int16

LN2 = math.log(2.0)
LOG1024 = math.log(1024.0)
EPS = 1e-5

OR = AluOpType.bitwise_or
AND = AluOpType.bitwise_and
ADD = AluOpType.add
SUB = AluOpType.subtract
MULT = AluOpType.mult
MAX = AluOpType.max
SHR = AluOpType.logical_shift_right

N_CORES = 8
BATCH = 262144
PER_CORE = BATCH // N_CORES  # 32768
F = 32  # grids per partition per tile
MOD_BIAS = -0.4999  # subtracted before the int16 convert in floor(c/15)
ACT_TABLE_BOTH = 6  # act_func_set_id of natural_log_exp_and_others


def _masks(nc, wp, es, nb8, F, Fa=None):
    """Unit masks through qx for BOTH tensors, ops interleaved pairwise.

    Each cross-engine hop (DVE -> Pool transpose -> DVE, DVE -> Act
    replicate -> DVE) is covered by the sibling tensor's independent
    work so the in-order DVE queue never stalls.  Also emits
    fw = 8*[cell empty] on the Act engine (consumed late, in the main
    stage).  Returns {key: (qx, col, fw)}.
    """
    if Fa is None:
        Fa = F
    t3s, rows, bcols, cols, boxes, rowxs, qs, qxs, fws = ({} for _ in range(9))

    for k, e in es.items():
        e4 = e.rearrange("p (f r c) -> p f r c", f=F, r=9, c=9)
        t3 = wp.tile([128, Fa * 27], U16, tag="t3", bufs=3, name=f"t3{k}")[:][
            :, : F * 27
        ]
        t3v = t3.rearrange("p (f r b) -> p f r b", f=F, r=9, b=3)
        nc.vector.tensor_tensor(t3v, e4[:, :, :, 0:3], e4[:, :, :, 3:6], op=OR)
        nc.vector.tensor_tensor(t3v, t3v, e4[:, :, :, 6:9], op=OR)
        t3s[k] = t3

    for k, e in es.items():
        e5 = e.rearrange("p (f b i c) -> p f b i c", f=F, b=3, i=3, c=9)
        bcol = wp.tile([128, Fa * 27], U16, tag="bcol", bufs=3, name=f"bcol{k}")[:][
            :, : F * 27
        ]
        bv = bcol.rearrange("p (f b c) -> p f b c", f=F, b=3, c=9)
        nc.vector.tensor_tensor(bv, e5[:, :, :, 0, :], e5[:, :, :, 1, :], op=OR)
        nc.vector.tensor_tensor(bv, bv, e5[:, :, :, 2, :], op=OR)
        bcols[k] = bcol

    for k in es:
        col = wp.tile([128, Fa * 9], U16, tag="col", bufs=3, name=f"col{k}")[:][
            :, : F * 9
        ]
        cv = col.rearrange("p (f c) -> p f c", f=F, c=9)
        bc3 = bcols[k].rearrange("p (f b c) -> p f b c", f=F, b=3, c=9)
        nc.vector.tensor_tensor(cv, bc3[:, :, 0, :], bc3[:, :, 1, :], op=OR)
        nc.vector.tensor_tensor(cv, cv, bc3[:, :, 2, :], op=OR)
        cols[k] = col

    for k in es:
        row = wp.tile([128, Fa * 9], U16, tag="row", bufs=3, name=f"row{k}")[:][
            :, : F * 9
        ]
        rv = row.rearrange("p (f r) -> p f r", f=F, r=9)
        t3b = t3s[k].rearrange("p (f r b) -> p f r b", f=F, r=9, b=3)
        nc.vector.tensor_tensor(rv, t3b[:, :, :, 0], t3b[:, :, :, 1], op=OR)
        nc.vector.tensor_tensor(rv, rv, t3b[:, :, :, 2], op=OR)
        rows[k] = row

    for k in es:
        # rowx[f, (b,i), k] = row[f, (b,i)]: replicate on Act (stride-0)
        rowx = wp.tile([128, Fa * 27], U16, tag="rowx", bufs=3, name=f"rowx{k}")[:][
            :, : F * 27
        ]
        rowxv = rowx.rearrange("p (u k) -> p u k", u=F * 9, k=3)
        nc.scalar.copy(rowxv, rows[k].unsqueeze(2).broadcast_to((128, F * 9, 3)))
        rowxs[k] = rowx

    for k in es:
        box = wp.tile([128, Fa * 9], U16, tag="box", bufs=3, name=f"box{k}")[:][
            :, : F * 9
        ]
        xv3 = box.rearrange("p (f b k) -> p f b k", f=F, b=3, k=3)
        bc4 = bcols[k].rearrange("p (f b k i) -> p f b k i", f=F, b=3, k=3, i=3)
        nc.vector.tensor_tensor(xv3, bc4[:, :, :, :, 0], bc4[:, :, :, :, 1], op=OR)
        nc.vector.tensor_tensor(xv3, xv3, bc4[:, :, :, :, 2], op=OR)
        boxes[k] = box.rearrange("p (g k) -> p g k", g=F * 3, k=3)

    for k in es:
        # q[f, (b,i), bc] = rowx | box (box bcast over i sits on a
        # middle dim, so every AP keeps a packed innermost -> 2x mode)
        q = wp.tile([128, Fa * 27], U16, tag="q", bufs=3, name=f"q{k}")[:][
            :, : F * 27
        ]
        qv = q.rearrange("p (g i k) -> p g i k", g=F * 3, i=3, k=3)
        nc.vector.tensor_tensor(
            qv,
            rowxs[k].rearrange("p (g i k) -> p g i k", g=F * 3, i=3, k=3),
            boxes[k].unsqueeze(2).broadcast_to((128, F * 3, 3, 3)),
            op=OR,
        )
        qs[k] = q

    for k in es:
        # qx[f, r, c] = q[f, r, bc(c)]: replicate 3x innermost on Pool
        # (stride-0 reads are legal there).  m = qx | col then runs as
        # one packed 2x DVE op instead of three 1x broadcast ops.
        qx = wp.tile([128, Fa * 81], U16, tag="qx", name=f"qx{k}")[:][
            :, : F * 81
        ]
        qxv = qx.rearrange("p (u i) -> p u i", u=F * 27, i=3)
        qu = qs[k].rearrange("p (u) -> p u", u=F * 27)
        nc.gpsimd.tensor_copy(qxv, qu.unsqueeze(2).broadcast_to((128, F * 27, 3)))
        qxs[k] = qx

    for k, e in es.items():
        # fw = relu(e/64 - 8): 1024 (empty) -> 8, every value bit -> 0.
        fw = wp.tile([128, Fa * 81], U16, tag="fw", name=f"fw{k}")[:][
            :, : F * 81
        ]
        nc.scalar.activation(
            fw, e[:], mybir.ActivationFunctionType.Relu, bias=nb8, scale=1.0 / 64.0
        )
        fws[k] = fw

    return {k: (qxs[k], cols[k], fws[k]) for k in es}


def _combine(nc, wp, qx_s, col_s, Fp, Fa):
    """m = qx | col on a sub-range of Fp grids (tile allocated at Fa)."""
    m_t = wp.tile([128, Fa * 81], U16, tag="m", bufs=2)
    m = m_t[:][:, : Fp * 81]
    mv3 = m.rearrange("p (f r c) -> p f r c", f=Fp, r=9, c=9)
    colb = col_s.rearrange("p (f c) -> p f c", f=Fp, c=9)
    nc.vector.tensor_tensor(
        mv3,
        qx_s.rearrange("p (f r c) -> p f r c", f=Fp, r=9, c=9),
        colb.unsqueeze(2).broadcast_to((128, Fp, 9, 9)),
        op=OR,
    )
    return m


def _entropy_u(nc, wp, m, fw_s, zb, Fp, Fa):
    """u(i16) with Ln(u+1) = per-cell entropy contribution.

    SWAR popcount of m's bits 1..9 (bit 10 never enters: the 0x155/0x55
    masks skip it), mod-15 digit sum (u16 multiply saturates on TRN2 so
    the wrap-multiply shortcut is unusable), fused with the own-cell
    empty gate fw.  Buffers A/B/C and m are reused in place across
    chain stages.
    """
    n = Fp * 81
    A = wp.tile([128, Fa * 81], U16, tag="A", bufs=2, name="A")[:][:, :n]
    B = wp.tile([128, Fa * 81], U16, tag="B", bufs=2, name="B")[:][:, :n]
    C = wp.tile([128, Fa * 81], U16, tag="C", bufs=2, name="C")[:][:, :n]

    h, g = A, B
    nc.vector.tensor_scalar(h, m, 1, 0x155, op0=SHR, op1=AND)
    nc.vector.tensor_scalar(g, m, 2, 0x55, op0=SHR, op1=AND)
    s = m  # m dead after h,g
    nc.vector.tensor_tensor(s, h, g, op=ADD)
    a2, c0 = A, B  # h,g consumed
    nc.vector.tensor_scalar(a2, s, 2, 0x33, op0=SHR, op1=AND)
    nc.vector.tensor_scalar(c0, s, 0x333, None, op0=AND)
    c = C
    nc.vector.tensor_tensor(c, c0, a2, op=ADD)
    # c = f0 + 16*f1 + 256*f2 with f0,f1<=4, f2<=1; popcount = c mod 15
    q15 = A.bitcast(I16)  # a2 consumed
    nc.vector.tensor_scalar(q15, c, 1.0 / 15.0, MOD_BIAS, op0=MULT, op1=ADD)
    v15 = B.bitcast(I16)  # c0 consumed
    nc.vector.tensor_scalar(v15, q15, 15, None, op0=MULT)
    t1 = m.bitcast(I16)  # s dead after a2,c0
    nc.vector.tensor_tensor(t1, c, v15, op=SUB)
    t = A.bitcast(I16)  # q15 dead
    nc.vector.tensor_tensor(t, t1, fw_s.bitcast(I16), op=SUB)
    # u = relu(-t) runs on the Activation engine (chain sink: it feeds
    # only the Ln on the same engine, so the DVE queue never waits)
    u = B.bitcast(I16)  # v15 dead
    nc.scalar.activation(u, t, mybir.ActivationFunctionType.Relu, bias=zb, scale=-1.0)
    return u


def _emit(tc, out_ap, gb_ap, ga_ap, n_grids, F):
    nc = tc.nc
    per_tile = 128 * F
    n_tiles = n_grids // per_tile

    # Pin the activation table that contains BOTH exp and ln: without
    # this the table-load pass alternates exp/ln tables (1.3us each).
    ld = mybir.InstLoadActFuncSet(
        name=nc.get_next_instruction_name(),
        act_func_set_id=ACT_TABLE_BOTH,
        ins=[],
        outs=[],
    )
    nc.scalar.add_instruction(ld)

    with ExitStack() as ctx:
        cp = ctx.enter_context(tc.tile_pool(name="const", bufs=1))
        iop = ctx.enter_context(tc.tile_pool(name="io", bufs=3))
        wp = ctx.enter_context(tc.tile_pool(name="work", bufs=4))
        accp = ctx.enter_context(tc.tile_pool(name="acc", bufs=3))

        enc_bias = cp.tile([128, 1], F32, tag="enc_bias")
        nc.vector.memset(enc_bias[:], LOG1024 + EPS)
        nb8 = cp.tile([128, 1], F32, tag="nb8")
        nc.vector.memset(nb8[:], -8.0)
        zb = cp.tile([128, 1], F32, tag="zb")
        nc.vector.memset(zb[:], 0.0)

        state = {}

        def pre(i):
            """DMA + encode + mask build through qx for tile i.

            Tile 0 runs in two half-F parts so the DVE starts working
            after half a DMA+Exp instead of a full one (pipeline head).
            """
            parts = (
                [(0, F)] if i > 0 else [(0, F // 8), (F // 8, F // 2), (F // 2, F)]
            )
            out = []
            for f0, f1 in parts:
                Fp = f1 - f0
                es = {}
                for key, src in (("b", gb_ap), ("a", ga_ap)):
                    x = iop.tile([128, F * 81], F32, tag="x", name=f"x{key}")
                    xs = x[:][:, : Fp * 81]
                    view = src[i * per_tile : (i + 1) * per_tile, :].rearrange(
                        "(p f) c -> p (f c)", p=128
                    )
                    nc.sync.dma_start(xs, view[:, f0 * 81 : f1 * 81])
                    e = wp.tile([128, F * 81], U16, tag="e", name=f"e{key}")
                    es[key] = e[:][:, : Fp * 81]
                    nc.scalar.activation(
                        es[key],
                        xs,
                        mybir.ActivationFunctionType.Exp,
                        bias=enc_bias[:],
                        scale=-LN2,
                    )
                out.append(((f0, f1), _masks(nc, wp, es, nb8, Fp, F)))
            state[i] = out

        def main(i):
            """Per-cell mask | col, SWAR chain, Ln, Pool fold for tile i.

            The last tile runs in two half-F parts so the pipeline drain
            tail (Ln + fold tree + out-DMA after the final DVE op) is
            half as long.
            """
            pre_parts = state.pop(i)
            work = []
            for (pf0, pf1), st in pre_parts:
                if i == n_tiles - 1 and pf1 - pf0 > F // 2:
                    mid = (pf0 + pf1) // 2
                    work.append(((pf0, mid), st, pf0))
                    work.append(((mid, pf1), st, pf0))
                else:
                    work.append(((pf0, pf1), st, pf0))
            for (f0, f1), st, base in work:
                Fp = f1 - f0
                tots = {}
                for key in ("b", "a"):
                    qx, col, fw = st[key]
                    qx_s = qx[:, (f0 - base) * 81 : (f1 - base) * 81]
                    col_s = col[:, (f0 - base) * 9 : (f1 - base) * 9]
                    fw_s = fw[:, (f0 - base) * 81 : (f1 - base) * 81]
                    m = _combine(nc, wp, qx_s, col_s, Fp, F)
                    u = _entropy_u(nc, wp, m, fw_s, zb, Fp, F)
                    lnv = wp.tile([128, F * 81], F32, tag="lnv", bufs=2)
                    lnv_s = lnv[:][:, : Fp * 81]
                    nc.scalar.activation(
                        lnv_s, u, mybir.ActivationFunctionType.Ln, bias=1.0
                    )
                    # Per-grid sum of the 81 ln values entirely on the
                    # Pool engine: in-place 81->27->9->3->1 fold tree
                    # (Pool only supports f32 arithmetic).
                    lv = lnv_s.rearrange("p (f c) -> p f c", f=Fp, c=81)
                    for width in (27, 9, 3, 1):
                        nc.gpsimd.tensor_tensor(
                            lv[:, :, 0:width],
                            lv[:, :, 0:width],
                            lv[:, :, width : 2 * width],
                            op=ADD,
                        )
                        nc.gpsimd.tensor_tensor(
                            lv[:, :, 0:width],
                            lv[:, :, 0:width],
                            lv[:, :, 2 * width : 3 * width],
                            op=ADD,
                        )
                    tots[key] = lv[:, :, 0]

                diff = accp.tile([128, F], F32, tag="diff")
                diff_s = diff[:][:, :Fp]
                nc.gpsimd.tensor_tensor(diff_s, tots["b"], tots["a"], op=SUB)
                nc.gpsimd.tensor_scalar(diff_s, diff_s, 1.0 / LN2, None, op0=MULT)
                out_view = out_ap[i * per_tile : (i + 1) * per_tile].rearrange(
                    "(p f) -> p f", p=128
                )
                nc.sync.dma_start(out_view[:, f0:f1], diff_s)

        # one-tile software pipeline skew: tile i's cross-engine mask
        # staging (Act transposes/replications) completes while the DVE
        # drains tile i-1's long chain, so the in-order DVE queue never
        # stalls on the Activation engine.
        for i in range(n_tiles + 1):
            if i < n_tiles:
                pre(i)
            if i >= 1:
                main(i - 1)


_PROGRAM_CACHE = {}


def _build_program():
    key = (PER_CORE, F)
    if key in _PROGRAM_CACHE:
        return _PROGRAM_CACHE[key]
    nc = bacc.Bacc("TRN2", target_bir_lowering=False, debug=False)
    gb = nc.dram_tensor("grid_before", [PER_CORE, 81], F32, kind="ExternalInput")
    ga = nc.dram_tensor("grid_after", [PER_CORE, 81], F32, kind="ExternalInput")
    out = nc.dram_tensor("out", [PER_CORE], F32, kind="ExternalOutput")
    with tile.TileContext(nc) as tc:
        _emit(tc, out.ap(), gb.ap(), ga.ap(), PER_CORE, F)
    nc.finalize()
    _PROGRAM_CACHE[key] = nc
    return nc


def run(grid_before, grid_after, trace=False, **trace_kwargs):
    gb = np.ascontiguousarray(
        np.asarray(grid_before, dtype=np.float32).reshape(BATCH, 81)
    )
    ga = np.ascontiguousarray(
        np.asarray(grid_after, dtype=np.float32).reshape(BATCH, 81)
    )
    nc = _build_program()
    in_maps = [
        {
            "grid_before": gb[k * PER_CORE : (k + 1) * PER_CORE],
            "grid_after": ga[k * PER_CORE : (k + 1) * PER_CORE],
        }
        for k in range(N_CORES)
    ]
    res = run_bass_kernel_spmd(
        nc, in_maps, list(range(N_CORES)), trace=trace, **trace_kwargs
    )
    out = np.concatenate([res.results[k]["out"] for k in range(N_CORES)])
    return out, res


def kernel(grid_before, grid_after):
    out, _ = run(grid_before, grid_after)
    return out


def bench(grid_before, grid_after, iters=12, warmup=3):
    """Time repeated executions with device-resident inputs.

    Mirrors bass2jax.run_bass_via_pjrt's shard_map structure but keeps the
    170MB of inputs on the devices between iterations, so the measured
    per-iteration wall time approximates kernel execution + dispatch.
    """
    import time

    import jax
    import concourse.mybir as mybir_
    from jax.sharding import Mesh, NamedSharding, PartitionSpec
    from jax.experimental.shard_map import shard_map
    from concourse.bass2jax import (
        _bass_exec_p,
        install_neuronx_cc_hook,
        partition_id_tensor,
    )

    install_neuronx_cc_hook()
    gb = np.ascontiguousarray(
        np.asarray(grid_before, dtype=np.float32).reshape(BATCH, 81)
    )
    ga = np.ascontiguousarray(
        np.asarray(grid_after, dtype=np.float32).reshape(BATCH, 81)
    )
    nc = _build_program()

    part_name = nc.partition_id_tensor.name if nc.partition_id_tensor else None
    in_names, out_names, out_avals, zero_outs = [], [], [], []
    for alloc in nc.m.functions[0].allocations:
        if not isinstance(alloc, mybir.MemoryLocationSet):
            continue
        name = alloc.memorylocations[0].name
        if alloc.kind == "ExternalInput":
            if name != part_name:
                in_names.append(name)
        elif alloc.kind == "ExternalOutput":
            out_names.append(name)
            shape = tuple(alloc.tensor_shape)
            dtype = mybir_.dt.np(alloc.dtype)
            out_avals.append(jax.core.ShapedArray(shape, dtype))
            zero_outs.append(np.zeros((N_CORES * shape[0], *shape[1:]), dtype))
    n_params = len(in_names)
    all_names = in_names + out_names
    if part_name is not None:
        all_names = all_names + [part_name]

    def _body(*args):
        operands = list(args)
        if part_name is not None:
            operands.append(partition_id_tensor())
        outs = _bass_exec_p.bind(
            *operands,
            out_avals=tuple(out_avals),
            in_names=tuple(all_names),
            out_names=tuple(out_names),
            lowering_input_output_aliases=(),
            sim_require_finite=True,
            sim_require_nnan=True,
            nc=nc,
        )
        return tuple(outs)

    devices = jax.devices()[:N_CORES]
    mesh = Mesh(np.asarray(devices), ("core",))
    spec = NamedSharding(mesh, PartitionSpec("core"))
    sharded = jax.jit(
        shard_map(
            _body,
            mesh=mesh,
            in_specs=(PartitionSpec("core"),) * (n_params + len(out_names)),
            out_specs=(PartitionSpec("core"),) * len(out_names),
            check_rep=False,
        ),
        keep_unused=True,
    )
    host_in = {"grid_before": gb, "grid_after": ga}
    dev_in = [jax.device_put(host_in[nm], spec) for nm in in_names]
    dev_zero = [jax.device_put(z, spec) for z in zero_outs]

    for _ in range(warmup):
        outs = sharded(*dev_in, *dev_zero)
    jax.block_until_ready(outs)
    t0 = time.perf_counter()
    for _ in range(iters):
        outs = sharded(*dev_in, *dev_zero)
    jax.block_until_ready(outs)
    t1 = time.perf_counter()
    per_iter_ns = (t1 - t0) / iters * 1e9
    out = np.asarray(outs[0])
    return per_iter_ns, out
